# revision 18
# baseline (speedup 1.0000x reference)
"""Self-contained Trainium2 Bass kernel for causal MHA.

Problem: B=2, S=2048, D=1024, H=16 heads of dim 64, fp32, causal softmax.
  out = softmax(mask(QK^T/8)) V W_0 + b_0 with QKV = X W_qkv + b_qkv.

Sharding: 8 NeuronCores = 2 batches x 4 head-groups (4 heads each),
tensor-parallel over heads, data-parallel over batch. Each core computes a
partial output projection for its 4 heads; host sums the 4 partials per
batch and adds the (bias-folded) output bias.

All matmul operands live in SBUF as bf16 (fp32 PSUM accumulation), which
halves HBM traffic / SBUF footprint, doubles DVE copy throughput, and
enables fast-weight-load on the PE. Accuracy stays ~1e-3 (budget 2e-2).

Device program per core:
  P1  qkT[c,s] = Wqk^T X^T (c = 4 heads' q rows then k rows), +bq on q rows
      (k bias dropped: it cancels in softmax over k).
  P2  V'' [s, 4*65]: V natural per head plus a ones column, so the
      attn.V matmul also produces softmax row sums for free.
  P3  per head, per 512-wide q superblock: S^T[k,q] = K^T.T @ Q^T computed
      transposed; causal mask on diagonal tiles added inside the matmul
      accumulation group via triangular bf16 factor matrices; exp on
      ScalarE (scale=1/8) -> P^T (bf16); ctx^T[65,512] += V''^T @ P^T over
      k blocks. Row 64 of ctx^T holds softmax denominators; normalize via
      K=1 broadcast matmul + reciprocal + multiply.
  P4  out[s,d] = ctxT.T @ W0 -> DRAM (bf16).

P3 is software-pipelined: the instruction stream is emitted with a 2-unit
skew (scores of unit i, ctx of unit i-2, exp of unit i-1) so the in-order
PE never stalls on the ACT exp. Next-round P1/P2 tiles are injected into
the pipeline drain at round boundaries to keep the PE fed.
"""
from contextlib import ExitStack

import numpy as np

import concourse.bass as bass
import concourse.mybir as mybir
import concourse.tile as tile
from concourse import bacc
from concourse.bass_utils import run_bass_kernel_spmd

F32 = mybir.dt.float32
F32R = mybir.dt.float32r
BF16 = mybir.dt.bfloat16
EXP = mybir.ActivationFunctionType.Exp
COPY = mybir.ActivationFunctionType.Copy

S, D, H, HD = 2048, 1024, 16, 64
HG = 4        # heads per core
NB = 4        # 512-wide q superblocks
KC = 8        # contraction chunks of 128 over D
NEG = -1e30


def _emit(tc, io):
    nc = tc.nc
    with ExitStack() as ctx:
        sb = ctx.enter_context(tc.tile_pool(name="sb", bufs=1))
        ps = ctx.enter_context(tc.tile_pool(name="ps", bufs=2, space="PSUM"))
        wk = ctx.enter_context(tc.tile_pool(name="wk", bufs=2))

        # ---- input DMAs (chunked so compute starts on first arrivals)
        wqk_sb = sb.tile([128, KC, 512], BF16, tag="wqk")
        wqk_r = io["wqk"].rearrange("(kc p) n -> kc p n", p=128)
        xt_sb = sb.tile([128, KC, S], BF16, tag="xt")
        xt_r = io["xt"].rearrange("(kc p) s -> kc p s", p=128)
        wv_sb = sb.tile([128, KC, 256], BF16, tag="wv")
        wv_r = io["wv"].rearrange("(kc p) n -> kc p n", p=128)
        for kc in range(KC):
            nc.sync.dma_start(out=wqk_sb[:, kc, :], in_=wqk_r[kc])
            nc.sync.dma_start(out=xt_sb[:, kc, 0:1024], in_=xt_r[kc][:, 0:1024])
            nc.scalar.dma_start(out=wv_sb[:, kc, :], in_=wv_r[kc])
        for kc in range(KC):
            nc.sync.dma_start(out=xt_sb[:, kc, 1024:], in_=xt_r[kc][:, 1024:])
        bq_sb = sb.tile([128, 2], F32, tag="bq")
        nc.gpsimd.dma_start(out=bq_sb, in_=io["bq2"])
        # 0/1 causal masks for the 4 diagonal offsets: mtri[t][k_local, q]
        # = 0 where k_local > q - 128*t else 1. Applied to P^T on gpsimd.
        mtri_sb = sb.tile([128, 4, 512], BF16, tag="mtri")
        nc.gpsimd.dma_start(out=mtri_sb, in_=io["mtri"])
        w0_sb = sb.tile([128, 2, D], BF16, tag="w0")
        w0_r = io["w0"].rearrange("(t p) n -> t p n", p=128)
        for t in range(2):
            nc.scalar.dma_start(out=w0_sb[:, t, :], in_=w0_r[t])

        qkT = sb.tile([128, 4, S], BF16, tag="qkT")  # rows: 0-255 q, 256-511 k
        vv = sb.tile([128, 16, HG * 65], BF16, tag="vv")
        ctxT = sb.tile([128, 2, S], BF16, tag="ctxT")
        ones1 = sb.tile([1, 64], F32R, tag="ones1")
        ones1f = sb.tile([1, 64], F32, tag="ones1f")
        nc.vector.memset(ones1f, 1.0)
        nc.vector.tensor_copy(ones1, ones1f)
        ones_col = sb.tile([128, HG, 1], BF16, tag="onescol")
        nc.vector.memset(ones_col, 1.0)

        def p1_tile(t, n):
            p1 = ps.tile([128, 512], F32, tag="strip")
            for kc in range(KC):
                nc.tensor.matmul(
                    p1,
                    lhsT=wqk_sb[:, kc, t * 128:(t + 1) * 128],
                    rhs=xt_sb[:, kc, n * 512:(n + 1) * 512],
                    start=(kc == 0), stop=(kc == KC - 1))
            dst = qkT[:, t, n * 512:(n + 1) * 512]
            if t < 2:
                nc.vector.tensor_scalar_add(dst, p1, bq_sb[:, t:t + 1])
            else:
                nc.vector.tensor_copy(dst, p1)

        def p2_tile(si):
            p2 = ps.tile([128, 512], F32, tag="strip")
            pp = p2[:, 0:256]
            for kc in range(KC):
                nc.tensor.matmul(
                    pp,
                    lhsT=xt_sb[:, kc, si * 128:(si + 1) * 128],
                    rhs=wv_sb[:, kc, :],
                    start=(kc == 0), stop=(kc == KC - 1))
            vsl = vv[:, si, :].rearrange("p (h c) -> p h c", c=65)
            nc.vector.tensor_copy(
                vsl[:, :, 0:64], pp.rearrange("p (h c) -> p h c", c=64))
            nc.vector.tensor_copy(vsl[:, :, 64:65], ones_col)

        # ---- P3 pipeline stages; per-(h,jp) state for the skewed stream
        st_sc = {}   # (h,jp) -> (sc tile, v0)
        st_pt = {}   # (h,jp) -> pt tile
        st_cps = {}  # h -> cps tile (per round, reset at round start)

        def s_unit(r, h, jp):
            t_q, t_k = h // 2, 2 + h // 2
            p0 = (h % 2) * 64
            sc = ps.tile([128, 1024], F32, tag="sc")
            v0 = [max(0, (jp * 2 + u) - 4 * r) * 128 * (1 - u)
                  for u in range(2)]
            for u in range(2):
                j = jp * 2 + u
                scu = sc[:, u * 512 + v0[u]:(u + 1) * 512]
                nc.tensor.matmul(
                    scu,
                    lhsT=qkT[p0:p0 + 64, t_k, j * 128:(j + 1) * 128],
                    rhs=qkT[p0:p0 + 64, t_q,
                            r * 512 + v0[u]:(r + 1) * 512],
                    start=True, stop=True)
            st_sc[(h, jp)] = (sc, v0)

        def e_unit(r, h, jp):
            sc, v0 = st_sc.pop((h, jp))
            pt = wk.tile([128, 1024], BF16, tag="pt", bufs=4)
            nc.scalar.activation(
                pt[:, v0[0]:], sc[:, v0[0]:], EXP, scale=0.125)
            for u in range(2):
                j = jp * 2 + u
                tt = j - 4 * r
                if tt >= 0:   # diagonal block: zero the causal upper part
                    ptu = pt[:, u * 512 + v0[u]:(u + 1) * 512]
                    eng = nc.gpsimd if u == 0 else nc.vector
                    eng.tensor_mul(ptu, ptu, mtri_sb[:, tt, v0[u]:])
            st_pt[(h, jp)] = (pt, v0)

        def c_unit(r, h, jp):
            p0 = (h % 2) * 64
            if jp == 0:
                st_cps[h] = ps.tile([65, 512], F32, tag="ctx", name="cps")
            cps = st_cps[h]
            pt, v0 = st_pt.pop((h, jp))
            for u in range(2):
                j = jp * 2 + u
                nc.tensor.matmul(
                    cps[:, v0[u]:],
                    lhsT=vv[:, j, :].rearrange(
                        "p (h c) -> p h c", c=65)[:, h, :],
                    rhs=pt[:, u * 512 + v0[u]:(u + 1) * 512],
                    start=(j == 0), stop=(j == 4 * r + 3))
            if jp == 2 * r + 1:
                # normalize by row sums (psum row 64)
                sm = wk.tile([1, 512], F32R, tag="sm", bufs=2)
                nc.vector.tensor_copy(sm, cps[64:65, :])
                bcp = ps.tile([64, 512], F32, tag="strip")
                nc.tensor.matmul(bcp, lhsT=ones1, rhs=sm,
                                 start=True, stop=True)
                rc = wk.tile([64, 512], F32, tag="rc", bufs=2)
                nc.vector.reciprocal(rc, bcp)
                nc.vector.tensor_mul(
                    ctxT[p0:p0 + 64, h // 2, r * 512:(r + 1) * 512],
                    cps[0:64, :], rc)

        def p4_tile(si, nn):
            po = ps.tile([128, 512], F32, tag="strip", name="po")
            for t in range(2):
                nc.tensor.matmul(
                    po,
                    lhsT=ctxT[:, t, si * 128:(si + 1) * 128],
                    rhs=w0_sb[:, t, nn * 512:(nn + 1) * 512],
                    start=(t == 0), stop=(t == 1))
            ob = wk.tile([128, 512], BF16, tag="ob", bufs=4)
            if (si + nn) % 2:
                nc.scalar.activation(ob, po, COPY)
            else:
                nc.vector.tensor_copy(ob, po)
            nc.sync.dma_start(
                out=io["out"][si * 128:(si + 1) * 128,
                              nn * 512:(nn + 1) * 512],
                in_=ob)

        # ---- global emission stream: P3 units with P1/P2/P4 "filler" work
        # sprinkled through the stream so ACT (exp) paces steady state and
        # the PE absorbs its per-unit slack.
        from collections import deque

        def pre_thunks(r):
            th = [lambda t=t: p1_tile(t, r) for t in (0, 2, 1, 3)]
            th += [lambda si=si: p2_tile(si) for si in range(4 * r, 4 * r + 4)]
            return th

        def p4_thunks(r):
            return [lambda si=si, nn=nn: p4_tile(si, nn)
                    for si in range(4 * r, 4 * r + 4) for nn in range(2)]

        fillers = deque()

        def sprinkle():
            if fillers:
                fillers.popleft()()

        for th in pre_thunks(0):
            th()
        for r in range(NB):
            units = [(h, jp) for h in range(HG) for jp in range(2 * (r + 1))]
            n = len(units)
            if r + 1 < NB:
                fillers.extend(pre_thunks(r + 1))
            for i, (h, jp) in enumerate(units):
                s_unit(r, h, jp)
                if i >= 3:
                    c_unit(r, *units[i - 3])
                if i >= 1:
                    e_unit(r, *units[i - 1])
                if fillers and (i % 2 == 0 or len(fillers) >= n - i):
                    sprinkle()
            # drain
            c_unit(r, *units[n - 3])
            sprinkle()
            e_unit(r, *units[n - 1])
            c_unit(r, *units[n - 2])
            sprinkle()
            c_unit(r, *units[n - 1])
            while fillers:
                sprinkle()
            fillers.extend(p4_thunks(r))
        while fillers:
            sprinkle()


def _declare_io(nc):
    return {
        "xt": nc.dram_tensor("xt", [D, S], BF16, kind="ExternalInput")[:, :],
        "wqk": nc.dram_tensor("wqk", [D, 512], BF16,
                              kind="ExternalInput")[:, :],
        "bq2": nc.dram_tensor("bq2", [128, 2], F32,
                              kind="ExternalInput")[:, :],
        "wv": nc.dram_tensor("wv", [D, 256], BF16, kind="ExternalInput")[:, :],
        "w0": nc.dram_tensor("w0", [256, D], BF16, kind="ExternalInput")[:, :],
        "mtri": nc.dram_tensor("mtri", [128, 4, 512], BF16,
                               kind="ExternalInput")[:, :, :],
        "out": nc.dram_tensor("out", [S, D], BF16,
                              kind="ExternalOutput")[:, :],
    }


_NC_CACHE = {}


def _build():
    if "nc" not in _NC_CACHE:
        nc = bacc.Bacc("TRN2", target_bir_lowering=False, debug=False,
                       num_devices=8)
        io = _declare_io(nc)
        with tile.TileContext(nc) as tc:
            _emit(tc, io)
        nc.compile()
        _NC_CACHE["nc"] = nc
    return _NC_CACHE["nc"]


def _causal_mask01():
    import ml_dtypes
    k = np.arange(128)[:, None]
    f = np.arange(512)[None, :]
    m = np.zeros((128, 4, 512), ml_dtypes.bfloat16)
    for t in range(4):
        m[:, t, :] = (k <= f - t * 128).astype(ml_dtypes.bfloat16)
    return m


def _core_inputs(X, W_qkv, b_qkv, W_0):
    import ml_dtypes
    bf = ml_dtypes.bfloat16
    mtri = _causal_mask01()
    maps = []
    for c in range(8):
        b, g = divmod(c, 4)
        cs = slice(g * 256, (g + 1) * 256)
        wqk = np.concatenate(
            [W_qkv[:, g * 256:(g + 1) * 256],
             W_qkv[:, 1024 + g * 256:1024 + (g + 1) * 256]], axis=1)
        maps.append({
            "xt": np.ascontiguousarray(X[b].T).astype(bf),
            "wqk": np.ascontiguousarray(wqk).astype(bf),
            "bq2": np.ascontiguousarray(b_qkv[cs].reshape(2, 128).T),
            "wv": np.ascontiguousarray(
                W_qkv[:, 2048 + g * 256:2048 + (g + 1) * 256]).astype(bf),
            "w0": np.ascontiguousarray(W_0[cs, :]).astype(bf),
            "mtri": mtri,
        })
    return maps


def kernel(X, W_qkv, b_qkv, W_0, b_0):
    X = np.asarray(X, np.float32)
    W_qkv = np.asarray(W_qkv, np.float32)
    b_qkv = np.asarray(b_qkv, np.float32)
    W_0 = np.asarray(W_0, np.float32)
    b_0 = np.asarray(b_0, np.float32)

    nc = _build()
    maps = _core_inputs(X, W_qkv, b_qkv, W_0)
    res = run_bass_kernel_spmd(nc, maps, core_ids=list(range(8))).results

    bias = b_qkv[2048:] @ W_0 + b_0   # V-bias folded (softmax rows sum to 1)
    out = np.zeros((2, S, D), np.float32)
    for c in range(8):
        out[c // 4] += res[c]["out"].astype(np.float32)
    out += bias[None, None, :]
    return out


# revision 34
# speedup vs baseline: 1.0187x; 1.0187x over previous
"""Self-contained Trainium2 Bass kernel for causal MHA.

Problem: B=2, S=2048, D=1024, H=16 heads of dim 64, fp32, causal softmax.
  out = softmax(mask(QK^T/8)) V W_0 + b_0 with QKV = X W_qkv + b_qkv.

Sharding: 8 NeuronCores = 2 batches x 4 head-groups (4 heads each),
tensor-parallel over heads, data-parallel over batch. Each core computes a
partial output projection for its 4 heads; host sums the 4 partials per
batch and adds the (bias-folded) output bias.

All matmul operands live in SBUF as bf16 (fp32 PSUM accumulation), which
halves HBM traffic / SBUF footprint, doubles DVE copy throughput, and
enables fast-weight-load on the PE. Accuracy stays ~1e-3 (budget 2e-2).

Device program per core:
  P1  qkT[c,s] = Wqk^T X^T (c = 4 heads' q rows then k rows), +bq on q rows
      (k bias dropped: it cancels in softmax over k).
  P2  V'' [s, 4*65]: V natural per head plus a ones column, so the
      attn.V matmul also produces softmax row sums for free.
  P3  per head, per 512-wide q superblock: S^T[k,q] = K^T.T @ Q^T computed
      transposed; causal mask on diagonal tiles added inside the matmul
      accumulation group via triangular bf16 factor matrices; exp on
      ScalarE (scale=1/8) -> P^T (bf16); ctx^T[65,512] += V''^T @ P^T over
      k blocks. Row 64 of ctx^T holds softmax denominators; normalize via
      K=1 broadcast matmul + reciprocal + multiply.
  P4  out[s,d] = ctxT.T @ W0 -> DRAM (bf16).

P3 is software-pipelined: the instruction stream is emitted with a 2-unit
skew (scores of unit i, ctx of unit i-2, exp of unit i-1) so the in-order
PE never stalls on the ACT exp. Next-round P1/P2 tiles are injected into
the pipeline drain at round boundaries to keep the PE fed.
"""
from contextlib import ExitStack

import numpy as np

import concourse.bass as bass
import concourse.mybir as mybir
import concourse.tile as tile
from concourse import bacc
from concourse.bass_utils import run_bass_kernel_spmd

F32 = mybir.dt.float32
F32R = mybir.dt.float32r
BF16 = mybir.dt.bfloat16
EXP = mybir.ActivationFunctionType.Exp
COPY = mybir.ActivationFunctionType.Copy

S, D, H, HD = 2048, 1024, 16, 64
HG = 4        # heads per core
NB = 4        # 512-wide q superblocks
KC = 8        # contraction chunks of 128 over D
NEG = -1e30


def _emit(tc, io):
    nc = tc.nc
    with ExitStack() as ctx:
        sb = ctx.enter_context(tc.tile_pool(name="sb", bufs=1))
        ps = ctx.enter_context(tc.tile_pool(name="ps", bufs=2, space="PSUM"))
        wk = ctx.enter_context(tc.tile_pool(name="wk", bufs=2))

        # ---- input DMAs (few large transfers; xt by q-superblock so
        # round-0 compute starts as soon as the first block lands)
        wqkv_sb = sb.tile([128, KC, 768], BF16, tag="wqkv")
        wqkv_r = io["wqkv"].rearrange("(kc p) n -> p kc n", p=128)
        xt_sb = sb.tile([128, KC, S], BF16, tag="xt")
        xt_r = {n: io["xt"][:, n * 512:(n + 1) * 512].rearrange(
            "(kc p) s -> p kc s", p=128) for n in range(4)}
        # first-needed quarters first: p1/p2 of round 0 consume wqkv + xt n=0
        for q in range(4):
            ks = slice(2 * q, 2 * q + 2)
            nc.sync.dma_start(out=wqkv_sb[:, ks, :], in_=wqkv_r[:, ks, :])
            nc.scalar.dma_start(out=xt_sb[:, ks, 0:512],
                                in_=xt_r[0][:, ks, :])
        for n in range(1, 4):
            eng = nc.sync if n % 2 == 0 else nc.scalar
            eng.dma_start(
                out=xt_sb[:, :, n * 512:(n + 1) * 512], in_=xt_r[n])
        bq_sb = sb.tile([128, 2], F32, tag="bq")
        nc.gpsimd.dma_start(out=bq_sb, in_=io["bq2"])
        # 0/1 causal masks for the 4 diagonal offsets: mtri[t][k_local, q]
        # = 0 where k_local > q - 128*t else 1. Applied to P^T on gpsimd.
        mtri_sb = sb.tile([128, 4, 512], BF16, tag="mtri")
        nc.gpsimd.dma_start(out=mtri_sb, in_=io["mtri"])
        w0_sb = sb.tile([128, 2, D], BF16, tag="w0")
        nc.scalar.dma_start(
            out=w0_sb, in_=io["w0"].rearrange("(t p) n -> p t n", p=128))

        qkT = sb.tile([128, 4, S], BF16, tag="qkT")  # rows: 0-255 q, 256-511 k
        vv = sb.tile([128, 16, HG * 65], BF16, tag="vv")
        ctxT = sb.tile([128, 2, S], BF16, tag="ctxT")
        ones1 = sb.tile([1, 64], F32R, tag="ones1")
        ones1f = sb.tile([1, 64], F32, tag="ones1f")
        nc.vector.memset(ones1f, 1.0)
        nc.vector.tensor_copy(ones1, ones1f)
        ones_col = sb.tile([128, HG, 1], BF16, tag="onescol")
        nc.vector.memset(ones_col, 1.0)

        def p1_tile(t, n):
            p1 = ps.tile([128, 512], F32, tag="strip")
            for kc in range(KC):
                nc.tensor.matmul(
                    p1,
                    lhsT=wqkv_sb[:, kc, t * 128:(t + 1) * 128],
                    rhs=xt_sb[:, kc, n * 512:(n + 1) * 512],
                    start=(kc == 0), stop=(kc == KC - 1))
            dst = qkT[:, t, n * 512:(n + 1) * 512]
            if t < 2:
                nc.vector.tensor_scalar_add(dst, p1, bq_sb[:, t:t + 1])
            else:
                nc.vector.tensor_copy(dst, p1)

        def p2_tile(si):
            p2 = ps.tile([128, 512], F32, tag="strip")
            pp = p2[:, 0:256]
            for kc in range(KC):
                nc.tensor.matmul(
                    pp,
                    lhsT=xt_sb[:, kc, si * 128:(si + 1) * 128],
                    rhs=wqkv_sb[:, kc, 512:768],
                    start=(kc == 0), stop=(kc == KC - 1))
            vsl = vv[:, si, :].rearrange("p (h c) -> p h c", c=65)
            nc.vector.tensor_copy(
                vsl[:, :, 0:64], pp.rearrange("p (h c) -> p h c", c=64))
            nc.vector.tensor_copy(vsl[:, :, 64:65], ones_col)

        # ---- P3 pipeline stages; per-(h,jp) state for the skewed stream
        st_sc = {}   # (h,jp) -> (sc tile, v0)
        st_pt = {}   # (h,jp) -> pt tile
        st_cps = {}  # h -> cps tile (per round, reset at round start)

        def s_unit(r, h, jp):
            # strip slot u=0 holds the MORE-masked j (2jp+1) so the column
            # restriction v0 (a strip-prefix cut) removes the most work;
            # slot u=1 holds 2jp, unrestricted (one contiguous exp).
            t_q, t_k = h // 2, 2 + h // 2
            p0 = (h % 2) * 64
            sc = ps.tile([128, 1024], F32, tag="sc")
            js = (jp * 2 + 1, jp * 2)
            v0 = (max(0, js[0] - 4 * r) * 128, 0)
            for u in range(2):
                scu = sc[:, u * 512 + v0[u]:(u + 1) * 512]
                nc.tensor.matmul(
                    scu,
                    lhsT=qkT[p0:p0 + 64, t_k, js[u] * 128:(js[u] + 1) * 128],
                    rhs=qkT[p0:p0 + 64, t_q,
                            r * 512 + v0[u]:(r + 1) * 512],
                    start=True, stop=True)
            st_sc[(h, jp)] = (sc, v0)

        def e_unit(r, h, jp):
            sc, v0 = st_sc.pop((h, jp))
            pt = wk.tile([128, 1024], BF16, tag="pt", bufs=4)
            nc.scalar.activation(
                pt[:, v0[0]:], sc[:, v0[0]:], EXP, scale=0.125)
            for u in range(2):
                tt = (jp * 2 + (1 - u)) - 4 * r
                if tt >= 0:   # diagonal block: zero the causal upper part
                    ptu = pt[:, u * 512 + v0[u]:(u + 1) * 512]
                    eng = nc.gpsimd if u == 0 else nc.vector
                    eng.tensor_mul(ptu, ptu, mtri_sb[:, tt, v0[u]:])
            st_pt[(h, jp)] = (pt, v0)

        def c_unit(r, h, jp):
            p0 = (h % 2) * 64
            if jp == 0:
                st_cps[h] = ps.tile([65, 512], F32, tag="ctx", name="cps")
            cps = st_cps[h]
            pt, v0 = st_pt.pop((h, jp))
            # u=1 (full-width j=2jp) first: the group-opening `start` must
            # zero the whole 512 columns before the restricted u=0 adds.
            for u in (1, 0):
                j = jp * 2 + (1 - u)
                nc.tensor.matmul(
                    cps[:, v0[u]:],
                    lhsT=vv[:, j, :].rearrange(
                        "p (h c) -> p h c", c=65)[:, h, :],
                    rhs=pt[:, u * 512 + v0[u]:(u + 1) * 512],
                    start=(jp == 0 and u == 1),
                    stop=(jp == 2 * r + 1 and u == 0))
            if jp == 2 * r + 1:
                # normalize by row sums (psum row 64)
                sm = wk.tile([1, 512], F32R, tag="sm", bufs=2)
                nc.vector.tensor_copy(sm, cps[64:65, :])
                bcp = ps.tile([64, 512], F32, tag="strip")
                nc.tensor.matmul(bcp, lhsT=ones1, rhs=sm,
                                 start=True, stop=True)
                rc = wk.tile([64, 512], F32, tag="rc", bufs=2)
                nc.vector.reciprocal(rc, bcp)
                nc.vector.tensor_mul(
                    ctxT[p0:p0 + 64, h // 2, r * 512:(r + 1) * 512],
                    cps[0:64, :], rc)

        def p4_tile(si, nn):
            po = ps.tile([128, 512], F32, tag="strip", name="po")
            for t in range(2):
                nc.tensor.matmul(
                    po,
                    lhsT=ctxT[:, t, si * 128:(si + 1) * 128],
                    rhs=w0_sb[:, t, nn * 512:(nn + 1) * 512],
                    start=(t == 0), stop=(t == 1))
            ob = wk.tile([128, 512], BF16, tag="ob", bufs=4)
            if (2 * si + nn) % 4 == 3:
                nc.scalar.activation(ob, po, COPY)
            else:
                nc.vector.tensor_copy(ob, po)
            nc.sync.dma_start(
                out=io["out"][si * 128:(si + 1) * 128,
                              nn * 512:(nn + 1) * 512],
                in_=ob)

        # ---- global emission stream: P3 units with P1/P2/P4 "filler" work
        # sprinkled through the stream so ACT (exp) paces steady state and
        # the PE absorbs its per-unit slack.
        from collections import deque

        def pre_thunks(r):
            th = [lambda t=t: p1_tile(t, r) for t in (0, 2, 1, 3)]
            th += [lambda si=si: p2_tile(si) for si in range(4 * r, 4 * r + 4)]
            return th

        def p4_thunks(r):
            return [lambda si=si, nn=nn: p4_tile(si, nn)
                    for si in range(4 * r, 4 * r + 4) for nn in range(2)]

        fillers = deque()

        def sprinkle():
            if fillers:
                fillers.popleft()()

        for th in pre_thunks(0):
            th()
        for r in range(NB):
            units = [(h, jp) for h in range(HG) for jp in range(2 * (r + 1))]
            n = len(units)
            if r + 1 < NB:
                fillers.extend(pre_thunks(r + 1))
            for i, (h, jp) in enumerate(units):
                s_unit(r, h, jp)
                if i >= 3:
                    c_unit(r, *units[i - 3])
                if i >= 1:
                    e_unit(r, *units[i - 1])
                if fillers and (i % 2 == 0 or len(fillers) >= n - i):
                    sprinkle()
            # drain
            c_unit(r, *units[n - 3])
            sprinkle()
            e_unit(r, *units[n - 1])
            c_unit(r, *units[n - 2])
            sprinkle()
            c_unit(r, *units[n - 1])
            while fillers:
                sprinkle()
            fillers.extend(p4_thunks(r))
        while fillers:
            sprinkle()


def _declare_io(nc):
    return {
        "xt": nc.dram_tensor("xt", [D, S], BF16, kind="ExternalInput")[:, :],
        "wqkv": nc.dram_tensor("wqkv", [D, 768], BF16,
                               kind="ExternalInput")[:, :],
        "bq2": nc.dram_tensor("bq2", [128, 2], F32,
                              kind="ExternalInput")[:, :],
        "w0": nc.dram_tensor("w0", [256, D], BF16, kind="ExternalInput")[:, :],
        "mtri": nc.dram_tensor("mtri", [128, 4, 512], BF16,
                               kind="ExternalInput")[:, :, :],
        "out": nc.dram_tensor("out", [S, D], BF16,
                              kind="ExternalOutput")[:, :],
    }


_NC_CACHE = {}


def _build():
    if "nc" not in _NC_CACHE:
        nc = bacc.Bacc("TRN2", target_bir_lowering=False, debug=False,
                       num_devices=8)
        io = _declare_io(nc)
        with tile.TileContext(nc) as tc:
            _emit(tc, io)
        nc.compile()
        _NC_CACHE["nc"] = nc
    return _NC_CACHE["nc"]


def _causal_mask01():
    import ml_dtypes
    k = np.arange(128)[:, None]
    f = np.arange(512)[None, :]
    m = np.zeros((128, 4, 512), ml_dtypes.bfloat16)
    for t in range(4):
        m[:, t, :] = (k <= f - t * 128).astype(ml_dtypes.bfloat16)
    return m


def _core_inputs(X, W_qkv, b_qkv, W_0):
    import ml_dtypes
    bf = ml_dtypes.bfloat16
    mtri = _causal_mask01()
    maps = []
    for c in range(8):
        b, g = divmod(c, 4)
        cs = slice(g * 256, (g + 1) * 256)
        wqkv = np.concatenate(
            [W_qkv[:, g * 256:(g + 1) * 256],
             W_qkv[:, 1024 + g * 256:1024 + (g + 1) * 256],
             W_qkv[:, 2048 + g * 256:2048 + (g + 1) * 256]], axis=1)
        maps.append({
            "xt": np.ascontiguousarray(X[b].T).astype(bf),
            "wqkv": np.ascontiguousarray(wqkv).astype(bf),
            "bq2": np.ascontiguousarray(b_qkv[cs].reshape(2, 128).T),
            "w0": np.ascontiguousarray(W_0[cs, :]).astype(bf),
            "mtri": mtri,
        })
    return maps


def kernel(X, W_qkv, b_qkv, W_0, b_0):
    X = np.asarray(X, np.float32)
    W_qkv = np.asarray(W_qkv, np.float32)
    b_qkv = np.asarray(b_qkv, np.float32)
    W_0 = np.asarray(W_0, np.float32)
    b_0 = np.asarray(b_0, np.float32)

    nc = _build()
    maps = _core_inputs(X, W_qkv, b_qkv, W_0)
    res = run_bass_kernel_spmd(nc, maps, core_ids=list(range(8))).results

    bias = b_qkv[2048:] @ W_0 + b_0   # V-bias folded (softmax rows sum to 1)
    out = np.zeros((2, S, D), np.float32)
    for c in range(8):
        out[c // 4] += res[c]["out"].astype(np.float32)
    out += bias[None, None, :]
    return out
